# revision 1
# baseline (speedup 1.0000x reference)
"""Jones congruence kernel (V_p = J1 @ V_m @ J2^T per baseline/time/freq) on 8 trn2 cores.

Sharding: time axis (64) split across 8 cores (8 t-steps each); every core
runs an identical program on its time slice. Simulated device time ~428us/core
(HBM roofline for the f32 streams is ~376us).

Per-core pipeline, 16 groups of 126 baselines:
  - V streams in as f32 via HWDGE (4MB/group, split hi/lo, prefetched 2 ahead)
    and is cast to fp16 by ScalarE *in place* (compacted into the upper half
    of the same buffer); T accumulates in the freed lower half.
  - j1/j2 are antenna gathers done as TensorEngine one-hot matmuls:
    onehot[128ant, 126bl]^T @ jones[128ant, sites] -> PSUM (f32), then
    ScalarE copies PSUM->SBUF casting to fp16 (512-site chunks, 2 PSUM bufs).
  - DVE does 19 fp16 plane ops (2x mode): 8 stage-1 muls + 4 adds into T,
    then 7 stage-2 muls whose products overwrite the dead j1s/j2s planes in a
    clobber-free order; GpSimd takes the 8th product (computed off T3, which
    stage 1 produces first) plus the final adds (fp16+fp16 -> f32) as four
    per-plane chunks interleaved with stage 2 so they start early. O streams out as f32 via
    HWDGE on the sync queue, ordered after the V prefetch so its dispatch
    wait cannot head-of-line block the next group's loads.
"""
import sys
sys.path.insert(0, "/opt/trn_rl_repo")
import numpy as np

NPOL, NANT, NBL, NTIMES, NFREQS = 2, 64, 2016, 64, 256
N_CORES = 8
T_LOC = NTIMES // N_CORES          # 8 timesteps per core
PLANE = T_LOC * NFREQS             # 2048 sites per (q, baseline) plane
GROUP = 126                        # baselines per tile group
N_GROUPS = NBL // GROUP            # 16
S_CHUNK = 512                      # psum gather chunk (one bank per q plane)
N_CHUNKS = PLANE // S_CHUNK        # 4

_cache = {}


def _split_excess_waits(nc, mybir):
    """Walrus in this env rejects >2 sem-wait conditions per instruction.
    Insert Drain clones carrying the excess waits immediately before."""
    fn = nc.m.functions[0]

    def walk(blocks):
        for bb in blocks:
            yield bb
            yield from walk(getattr(bb, "blocks", None) or [])

    ctr = [0]
    for bb in walk(fn.blocks):
        newlist = []
        for ins in bb.instructions:
            si = ins.sync_info
            if si is not None and si.on_wait and len(si.on_wait) > 1:
                waits = list(si.on_wait)
                while len(waits) > 1:
                    chunk, waits = waits[:1], waits[1:]
                    d = mybir.InstNoOp(
                        name=f"waitsplit-{ctr[0]}",
                        engine=ins.engine,
                        ins=[],
                        outs=[],
                        sync_info=mybir.SyncInfo(on_wait=chunk, on_update=[]),
                    )
                    ctr[0] += 1
                    newlist.append(d)
                si.on_wait = waits
            newlist.append(ins)
        bb.instructions = newlist


def _build():
    import concourse.bass as bass
    import concourse.tile as tile
    from concourse import mybir
    from contextlib import ExitStack

    f32, f16 = mybir.dt.float32, mybir.dt.float16
    # no SWDGE (gpsimd) DMAs are issued -> shrink its SBUF scratch carveout
    nc = bass.Bass("TRN2", target_bir_lowering=False, debug=False,
                   dynamic_dma_scratch_size=2048)
    V = nc.dram_tensor("V", [NBL, 4, PLANE], f32, kind="ExternalInput").ap()
    J = nc.dram_tensor("J", [NANT, 4, PLANE], f16, kind="ExternalInput").ap()
    W = nc.dram_tensor("W", [128, 2 * NBL], f16, kind="ExternalInput").ap()
    O = nc.dram_tensor("O", [NBL, 4, PLANE], f32, kind="ExternalOutput").ap()

    with tile.TileContext(nc) as tc:
        with ExitStack() as ctx:
            fixp = ctx.enter_context(tc.tile_pool(name="fix", bufs=1))
            viop = ctx.enter_context(tc.tile_pool(name="vio", bufs=2))
            outp = ctx.enter_context(tc.tile_pool(name="out", bufs=2))
            jp = ctx.enter_context(tc.tile_pool(name="jp", bufs=2))
            scp = ctx.enter_context(tc.tile_pool(name="scp", bufs=2))
            pp = ctx.enter_context(tc.tile_pool(name="pp", bufs=2, space="PSUM"))

            # jones moving table [128, 4, 2048] fp16 (rows 64-127 zero)
            jt = fixp.tile([128, 4, PLANE], f16, name="jt")
            nc.sync.dma_start(jt[:NANT], J[:])
            nc.vector.memset(jt[NANT:], 0.0)
            # software-pipelined V + one-hot-weight prefetch, depth 2
            vtiles = {}
            wtiles = {}

            def vin(g):
                n0 = g * GROUP
                vtiles[g] = viop.tile([GROUP, 4, PLANE], f32, tag="v32",
                                      name=f"v32_{g}")
                # hi half first: the hi cast (and stage 1) unblock sooner
                nc.sync.dma_start(vtiles[g][:, 2:4, :], V[n0:n0 + GROUP, 2:4])
                nc.sync.dma_start(vtiles[g][:, 0:2, :], V[n0:n0 + GROUP, 0:2])
                w1t = jp.tile([128, GROUP], f16, tag="w1", name=f"w1_{g}")
                w2t = jp.tile([128, GROUP], f16, tag="w2", name=f"w2_{g}")
                nc.sync.dma_start(w1t[:], W[:, n0:n0 + GROUP])
                nc.sync.dma_start(w2t[:], W[:, NBL + n0:NBL + n0 + GROUP])
                wtiles[g] = (w1t, w2t)

            vin(0)
            vin(1)

            # fp16 views carved inside the 32KB/partition f32 buffers:
            #  - v16 (cast V) lives in v32 bytes 16-32K, plane order p2,p3,p0,p1
            #    (v32's last reader is then stage 1 -> short V-refill chain)
            #  - T[a,c] accumulates in v32 bytes 0-16K (planes 0-1 region,
            #    dead after the lo cast); odd products use a 4KB scratch plane
            V16_OFF = {2: 0, 3: PLANE, 0: 2 * PLANE, 1: 3 * PLANE}

            def v16(v32, p):
                flat = v32[:, 2:4, :].bitcast(f16).rearrange("p a s -> p (a s)")
                return flat[:, V16_OFF[p]:V16_OFF[p] + PLANE]

            def tpl(v32, q):
                flat = v32[:, 0:2, :].bitcast(f16).rearrange("p a s -> p (a s)")
                return flat[:, q * PLANE:(q + 1) * PLANE]

            for g in range(N_GROUPS):
                n0 = g * GROUP
                v32 = vtiles.pop(g)
                ot = outp.tile([GROUP, 4, PLANE], f32, tag="ot", name=f"ot_{g}")

                # --- gather j1/j2 via one-hot matmuls, S_CHUNK sites/bank ---
                # ACT order: j1 copies, V casts, j2 copies — stage 1's inputs
                # (j1 + cast-lo) land earliest; j2 copies overlap stage 1.
                j1s = jp.tile([GROUP, 4, PLANE], f16, tag="j1", name="j1s")
                j2s = jp.tile([GROUP, 4, PLANE], f16, tag="j2", name="j2s")
                w1, w2 = wtiles.pop(g)

                def gather(js, w):
                    for c in range(N_CHUNKS):
                        s0 = c * S_CHUNK
                        pj = pp.tile([GROUP, 4, S_CHUNK], f32, tag="pj", name="pj")
                        for q in range(4):
                            nc.tensor.matmul(pj[:, q, :], w, jt[:, q, s0:s0 + S_CHUNK])
                        nc.scalar.copy(js[:, :, s0:s0 + S_CHUNK], pj[:])

                gather(j1s, w1[:])

                # compacting in-buffer casts. ORDER IS MANDATORY: the hi cast
                # reads f32 planes 2-3 (bytes 16-32K) and writes 16-24K
                # (forward-safe); only then may the lo cast overwrite 24-32K.
                # v16 layout (see V16_OFF): [p2|p3|p0|p1], each pair adjacent.
                vflat = v32[:, 2:4, :].bitcast(f16).rearrange("p a s -> p (a s)")
                nc.scalar.copy(vflat[:, 0:2 * PLANE].rearrange(
                    "p (a s) -> p a s", s=PLANE), v32[:, 2:4, :])
                nc.scalar.copy(vflat[:, 2 * PLANE:4 * PLANE].rearrange(
                    "p (a s) -> p a s", s=PLANE), v32[:, 0:2, :])

                gather(j2s, w2[:])

                # --- stage 1: T[a,c] = j1[a,0]*V[0,c] + j1[a,1]*V[1,c] ---
                # q order 3,0,2,1: T3 first so gpsimd's stage-2 product can
                # start while the rest of stage 1 runs; then T0/T2/T1 in the
                # order DVE's stage-2 consumes them.
                for a, cc in ((1, 1), (0, 0), (1, 0), (0, 1)):
                    q = 2 * a + cc
                    sp = scp.tile([GROUP, PLANE], f16, tag="s", name="s")
                    nc.vector.tensor_mul(tpl(v32, q),
                                         j1s[:, 2 * a + 0, :], v16(v32, 0 + cc))
                    nc.vector.tensor_mul(sp[:],
                                         j1s[:, 2 * a + 1, :], v16(v32, 2 + cc))
                    nc.vector.tensor_add(tpl(v32, q), tpl(v32, q), sp[:])

                # --- stage 2: O[a,d] = T[a,0]*j2[d,0] + T[a,1]*j2[d,1] ---
                # p0[(a,d)] -> j1s plane 2a+d (j1s dead after stage 1);
                # p1[(a,d)] -> j2s planes / scratch, ordered so every gathered
                # j2s plane is fully consumed before being overwritten, and so
                # the Pool adds can start early (plane-2 add after 4 muls).
                nc.vector.tensor_mul(j1s[:, 0, :], tpl(v32, 0), j2s[:, 0, :])  # p0 00
                nc.vector.tensor_mul(j1s[:, 1, :], tpl(v32, 0), j2s[:, 2, :])  # p0 01
                nc.vector.tensor_mul(j1s[:, 2, :], tpl(v32, 2), j2s[:, 0, :])  # p0 10
                sp2 = scp.tile([GROUP, PLANE], f16, tag="s", name="s2")
                # p1 10 on gpsimd: overlaps DVE's stage-2 and feeds its own add
                nc.gpsimd.tensor_mul(sp2[:], tpl(v32, 3), j2s[:, 1, :])
                nc.gpsimd.tensor_add(ot[:, 2, :], j1s[:, 2, :], sp2[:])
                nc.vector.tensor_mul(j2s[:, 0, :], tpl(v32, 1), j2s[:, 1, :])  # p1 00
                nc.gpsimd.tensor_add(ot[:, 0, :], j1s[:, 0, :], j2s[:, 0, :])
                nc.vector.tensor_mul(j2s[:, 1, :], tpl(v32, 1), j2s[:, 3, :])  # p1 01
                nc.gpsimd.tensor_add(ot[:, 1, :], j1s[:, 1, :], j2s[:, 1, :])
                nc.vector.tensor_mul(j1s[:, 3, :], tpl(v32, 2), j2s[:, 2, :])  # p0 11
                nc.vector.tensor_mul(j2s[:, 3, :], tpl(v32, 3), j2s[:, 3, :])  # p1 11
                nc.gpsimd.tensor_add(ot[:, 3, :], j1s[:, 3, :], j2s[:, 3, :])
                if g + 2 < N_GROUPS:
                    vin(g + 2)
                if g == N_GROUPS - 1:
                    # shorten the tail: ship each plane as soon as it is added
                    nc.sync.dma_start(O[n0:n0 + GROUP, 2:3], ot[:, 2:3, :])
                    nc.sync.dma_start(O[n0:n0 + GROUP, 0:1], ot[:, 0:1, :])
                    nc.sync.dma_start(O[n0:n0 + GROUP, 1:2], ot[:, 1:2, :])
                    nc.sync.dma_start(O[n0:n0 + GROUP, 3:4], ot[:, 3:4, :])
                else:
                    nc.sync.dma_start(O[n0:n0 + GROUP], ot[:])

    _split_excess_waits(nc, mybir)
    return nc


def _prep_inputs(V_m, jones, ant1, ant2):
    """Per-core input maps: V/J time-sliced, one-hot weights replicated."""
    wt = np.zeros((128, 2 * NBL), dtype=np.float16)
    wt[ant1, np.arange(NBL)] = 1.0
    wt[ant2, NBL + np.arange(NBL)] = 1.0
    in_maps = []
    for k in range(N_CORES):
        t0 = k * T_LOC
        vk = np.ascontiguousarray(
            V_m[:, :, :, t0:t0 + T_LOC, :].transpose(2, 0, 1, 3, 4)
        ).reshape(NBL, 4, PLANE)
        jk = np.ascontiguousarray(
            jones[:, :, :, t0:t0 + T_LOC, :].transpose(2, 0, 1, 3, 4)
        ).astype(np.float16).reshape(NANT, 4, PLANE)
        in_maps.append({"V": vk, "J": jk, "W": wt})
    return in_maps


def kernel(V_m, jones, ant1, ant2):
    from concourse.bass_utils import run_bass_kernel_spmd

    V_m = np.asarray(V_m, dtype=np.float32)
    jones = np.asarray(jones, dtype=np.float32)
    a1 = np.asarray(ant1).astype(np.int64)
    a2 = np.asarray(ant2).astype(np.int64)

    if "nc" not in _cache:
        _cache["nc"] = _build()
    nc = _cache["nc"]

    in_maps = _prep_inputs(V_m, jones, a1, a2)
    res = run_bass_kernel_spmd(nc, in_maps, list(range(N_CORES)))
    out = np.empty((NPOL, NPOL, NBL, NTIMES, NFREQS), dtype=np.float32)
    for k in range(N_CORES):
        t0 = k * T_LOC
        out[:, :, :, t0:t0 + T_LOC, :] = res.results[k]["O"].reshape(
            NBL, NPOL, NPOL, T_LOC, NFREQS).transpose(1, 2, 0, 3, 4)
    return out

